# revision 84
# baseline (speedup 1.0000x reference)
"""Multi-head attention (B=2, N=2048, D=1024, H=16, d_k=d_v=64) on 8
TRN2 NeuronCores.

Sharding: data parallel over batch (2) x tensor parallel over head
groups (4 heads per core). Each core computes its 4 heads' attention
plus the partial output projection (Wp rows for those heads); the host
sums the 4 partials per batch and adds the residual.

Design notes (vs the 257us baseline; ~186us measured):
  - attnv matmuls ride TWO slots behind their scores (FLUSH_DEPTH=2) so
    the exp engines have a full extra slot of slack before the in-order
    PE queue can block on them — worth ~20us alone.
  - the output projection also runs fp8 DoubleRow: outT is stored fp8
    as [dim, g2, seq] so one matmul contracts both 128-dim chunks; Wp is
    host-prescaled by 32 and evictions scale by 1/1024.
  - q/k/v projections run in fp8e4m3 with DoubleRow (contraction 256 per
    matmul, chunk-pair 3D APs over [128, 8, cols] tiles), halving their
    matmul count; weights are host-prescaled by 32 to dodge e4m3
    denormals and Wp is pre-divided by 32 to compensate.
  - exp is split between ScalarE (true exp, 10/16 slots) and VectorE
    (Schraudolph int16-bit-trick exp: bits = round(A*psum + 16250.4)
    viewed as bf16, ~3% max rel err, 6/16 slots, engine-alternating) so
    ACT is no longer a ~143us serial bottleneck.
  - softmax reciprocal: denominators are staged through an SBUF->SBUF
    reshape DMA into [128, 8] so one 128-lane reciprocal replaces the
    pathological [1,512] 1-lane reciprocals (53us -> ~1us), then one
    DRAM hop for the partition broadcast.
  - loads shrink to 3.25MB (fp8), issued from BOTH sync and scalar DMA
    queues; dummy matmuls on scratch SBUF keep the PE busy through the
    load wait so HAM un-throttles to 2.4 GHz by ~11us.
  - attention blocks alternate g2 (head-pair) per qb so projection and
    normalization spread evenly; proj matmuls and leftover chains ride
    in the slack via a fractionally-paced side queue.
"""
import numpy as np

import concourse.bass as bass
import concourse.tile as tile
from concourse import mybir
from concourse.vector_clock import ScopedClock

f32 = mybir.dt.float32
f32r = mybir.dt.float32r
bf16 = mybir.dt.bfloat16
i16 = mybir.dt.int16
f8 = mybir.dt.float8e4

B, N, D = 2, 2048, 1024
H, DK = 16, 64
HPC = 4          # heads per core
GCOLS = HPC * DK  # 256 weight columns per core
NCORES = 8
P = 128
NKB = N // P      # 16 key blocks
NQB = N // 512    # 4 query blocks of 512
NDMC = D // P     # 8 d_model chunks

# q/k/v weights and x are fed to the PE in fp8e4m3 with the weights
# pre-scaled by 32 (avoids e4m3 denormals); scores come out as
# 1024*(q.k) and Wp is pre-divided by 32 to undo the v scaling.
QK_SCALE = 1024.0
# Schraudolph fast-exp constants (DVE int16 trick), calibrated on HW:
# bits = round_i16(EXP_A * psum + EXP_B); bits viewed as bf16 ~= exp(s/8)
EXP_A = 0.125 * 1.4426950408889634 * 128.0 / QK_SCALE
EXP_B = 16256.0 - 5.6
ACT_SCALE = 0.125 / QK_SCALE
# slots (by kb) whose exp runs on DVE instead of ACT, alternating so
# consecutive slots' exps pipeline across the two engines; kb 0-1 stay
# on ACT so DVE is free to release the po psum at block boundaries.
DVE_KBS_EVEN = frozenset({2, 4, 6, 8, 10, 12})
DVE_KBS_ODD = frozenset({3, 5, 7, 9, 11, 13})

_cache = {}
_last_results = None


# ---------------------------------------------------------------------------
# Workarounds for this walrus build: max ONE sync wait per instruction.
# ---------------------------------------------------------------------------
_ws_counter = [0]


def _split_multi_waits(nc, limit=1):
    for f in nc.m.functions:
        for bb in f.blocks:
            new = []
            changed = False
            for inst in bb.instructions:
                si = inst.sync_info
                waits = list(si.on_wait) if si is not None and si.on_wait else []
                if len(waits) > limit:
                    changed = True
                    extra = waits[:-limit]
                    for i in range(0, len(extra), limit):
                        _ws_counter[0] += 1
                        nop = mybir.InstNoOp(
                            name=f"I-waitsplit-{_ws_counter[0]}", ins=[], outs=[]
                        )
                        nop.engine = inst.engine
                        nop.sync_info = mybir.SyncInfo(
                            on_wait=extra[i : i + limit], on_update=[]
                        )
                        new.append(nop)
                    si.on_wait = waits[-limit:]
                    inst.sync_info = si
                new.append(inst)
            if changed:
                bb.instructions = new


def _patched_drain_and_barrier(self, tick_clock, wait_clock):
    nc = self.nc
    drain_inst = nc.sync.drain()
    wait_clock.add_sem_waits(
        drain_inst.ins, ScopedClock({None: tick_clock.global_clock})
    )
    si = drain_inst.ins.sync_info
    if si is not None and si.on_wait is not None and len(si.on_wait) > 1:
        waits = list(si.on_wait)
        si.on_wait = waits[:1]
        drain_inst.ins.sync_info = si
        for i in range(1, len(waits)):
            extra = nc.sync.drain()
            esi = extra.ins.sync_info
            if esi is None:
                esi = mybir.SyncInfo(on_wait=[], on_update=[])
            esi.on_wait = waits[i : i + 1]
            extra.ins.sync_info = esi
    nc.all_engine_barrier()
    assert self.sems is not None
    popped = nc._tile_sem_poison_stack.pop()
    assert popped is self._sem_poison
    nc.clear_and_free_semaphores(list(self.sems.allocated().values()))
    nc.all_engine_barrier()


tile.TileContext._drain_and_barrier = _patched_drain_and_barrier


# ---------------------------------------------------------------------------
# Kernel build
# ---------------------------------------------------------------------------
def _build():
    nc = bass.Bass()
    # fp8 operands, chunk-major [128, 8 d-chunks, cols] for DoubleRow
    xtf = nc.dram_tensor("xtf", [P, NDMC, N], f8, kind="ExternalInput")
    wqkf = nc.dram_tensor("wqkf", [P, NDMC, 2 * GCOLS], f8,
                          kind="ExternalInput")
    wvf = nc.dram_tensor("wvf", [P, NDMC, GCOLS], f8, kind="ExternalInput")
    wpf = nc.dram_tensor("wpf", [P, 2, D], f8, kind="ExternalInput")
    ones = nc.dram_tensor("ones", [1, P], f32r, kind="ExternalInput")
    pout = nc.dram_tensor("pout", [N, D], bf16, kind="ExternalOutput")

    with tile.TileContext(nc) as tc:
        import contextlib

        with contextlib.ExitStack() as ctx:
            sbX = ctx.enter_context(tc.tile_pool(name="sbX", bufs=1))
            sbW = ctx.enter_context(tc.tile_pool(name="sbW", bufs=1))
            sbQK = ctx.enter_context(tc.tile_pool(name="sbQK", bufs=1))
            sbV = ctx.enter_context(tc.tile_pool(name="sbV", bufs=1))
            sbO = ctx.enter_context(tc.tile_pool(name="sbO", bufs=1))
            sbA = ctx.enter_context(tc.tile_pool(name="sbA", bufs=6))
            sbR = ctx.enter_context(tc.tile_pool(name="sbR", bufs=3))
            sbP = ctx.enter_context(tc.tile_pool(name="sbP", bufs=3))
            drS = ctx.enter_context(tc.tile_pool(name="drS", bufs=4,
                                                  space="DRAM"))
            psS = ctx.enter_context(tc.tile_pool(name="psS", bufs=2, space="PSUM"))
            psO = ctx.enter_context(tc.tile_pool(name="psO", bufs=1, space="PSUM"))
            psA = ctx.enter_context(tc.tile_pool(name="psA", bufs=2, space="PSUM"))

            # ---- loads ----------------------------------------------------
            # sync engine: ones, wqkf pairs, xtf col-waves 0 and 2.
            # scalar engine: wvf, xtf col-wave 1, warm, wp.
            ones_sb = sbW.tile([1, P], f32r, tag="ones")
            nc.sync.dma_start(out=ones_sb[:], in_=ones[:])

            wqkf_sb = sbW.tile([P, NDMC, 2 * GCOLS], f8, tag="wqkf")
            for cc in range(4):
                nc.sync.dma_start(out=wqkf_sb[:, 2 * cc : 2 * cc + 2, :],
                                  in_=wqkf[:, 2 * cc : 2 * cc + 2, :])
            xtf_sb = sbX.tile([P, NDMC, N], f8, tag="xtf")
            for cc in range(4):
                nc.sync.dma_start(
                    out=xtf_sb[:, 2 * cc : 2 * cc + 2, 0:512],
                    in_=xtf[:, 2 * cc : 2 * cc + 2, 0:512])

            wvf_sb = sbW.tile([P, NDMC, GCOLS], f8, tag="wvf")
            for half in range(2):
                nc.scalar.dma_start(out=wvf_sb[:, 4 * half : 4 * half + 4, :],
                                    in_=wvf[:, 4 * half : 4 * half + 4, :])
            for cc in range(4):
                nc.scalar.dma_start(
                    out=xtf_sb[:, 2 * cc : 2 * cc + 2, 512:1024],
                    in_=xtf[:, 2 * cc : 2 * cc + 2, 512:1024])

            # Pre-warm the exp table (~2.7us ACT table load) AFTER the hot
            # scalar-queue loads so it doesn't delay them, but well before
            # the first real exp.
            warm = sbR.tile([1, 2], f32, tag="warm")
            nc.scalar.activation(
                warm[:], ones_sb[0:1, 0:2], mybir.ActivationFunctionType.Exp
            )

            for cc in range(4):
                nc.sync.dma_start(
                    out=xtf_sb[:, 2 * cc : 2 * cc + 2, 1024:N],
                    in_=xtf[:, 2 * cc : 2 * cc + 2, 1024:N])

            wpf_sb = sbW.tile([P, 2, D], f8, tag="wpf")
            for g2 in range(2):
                nc.scalar.dma_start(out=wpf_sb[:, g2 : g2 + 1, :],
                                    in_=wpf[:, g2 : g2 + 1, :])

            # Dummy matmuls on scratch SBUF keep the PE busy through the
            # load wait so HAM un-throttles (K=8/8) before the real chains;
            # without them everything before ~20us runs at 1.2 GHz.
            wscr = sbW.tile([P, 512], bf16, tag="wscr")
            nc.vector.memset(wscr[:], 1.0)
            pscr = psA.tile([P, 512], f32, tag="pacc", name="pwarm")
            for _ in range(32):
                nc.tensor.matmul(pscr[:], wscr[:, 0:P], wscr[:],
                                 start=True, stop=True)

            qT = [sbQK.tile([P, N], bf16, tag=f"qT{g2}", name=f"qT{g2}")
                  for g2 in range(2)]
            kT = [sbQK.tile([P, N], bf16, tag=f"kT{g2}", name=f"kT{g2}")
                  for g2 in range(2)]
            vaug = [sbV.tile([P, NKB, 2, 65], bf16, tag=f"vaug{g2}",
                             name=f"vaug{g2}") for g2 in range(2)]
            for g2 in range(2):
                nc.vector.memset(vaug[g2][:, :, :, 64:65], 1.0)
            # normalized attention output, fp8, [dim-of-g2, g2, seq] so the
            # proj matmul can pair the two g2 chunks with DoubleRow.
            outT3 = sbO.tile([P, 2, N], f8, tag="outT3", name="outT3")

            # ---- chain thunk builders (fp8 DoubleRow: 4 chunk-pair MMs) --
            def qk_chain_thunks(dst, which, g2, qb):
                # which: 0 = wq (cols 0:256), 1 = wk (cols 256:512)
                base = which * GCOLS + g2 * P
                st = {}
                def mm(cc):
                    if cc == 0:
                        st["p"] = psA.tile([P, 512], f32, tag="pacc",
                                           name=f"pqk{which}_{g2}_{qb}")
                    nc.tensor.matmul(
                        st["p"][:],
                        wqkf_sb[:, 2 * cc : 2 * cc + 2, base : base + P],
                        xtf_sb[:, 2 * cc : 2 * cc + 2,
                               qb * 512 : (qb + 1) * 512],
                        start=(cc == 0), stop=(cc == 3),
                        perf_mode=mybir.MatmulPerfMode.DoubleRow,
                    )
                def evict():
                    nc.vector.tensor_copy(
                        dst[g2][:, qb * 512 : (qb + 1) * 512], st["p"][:]
                    )
                return [lambda cc=cc: mm(cc) for cc in range(4)] + [evict]

            def v_chain_thunks(kb):
                st = {}
                def mm(cc):
                    if cc == 0:
                        st["p"] = psA.tile([P, 2, P], f32, tag="pacc",
                                           name=f"pv{kb}")
                    nc.tensor.matmul(
                        st["p"][:],
                        xtf_sb[:, 2 * cc : 2 * cc + 2, kb * P : (kb + 1) * P],
                        wvf_sb[:, 2 * cc : 2 * cc + 2, :],
                        start=(cc == 0), stop=(cc == 3),
                        perf_mode=mybir.MatmulPerfMode.DoubleRow,
                    )
                def ev(g2):
                    nc.vector.tensor_copy(
                        vaug[g2][:, kb, :, 0:64], st["p"][:, g2, :]
                    )
                return ([lambda cc=cc: mm(cc) for cc in range(4)]
                        + [lambda: ev(0), lambda: ev(1)])

            def proj_thunks(sb):
                # per seq block sb: one DoubleRow matmul per 512-col half
                # (contracts both g2 chunks at once); psum holds 1024*pout
                # so the eviction scales by 1/1024.  Evictions split ACT/DVE.
                ot = sbP.tile([P, D], bf16, tag="pout", name=f"ot{sb}")
                st = {}
                def mk(half):
                    def mm():
                        st[half] = psA.tile([P, 512], f32, tag="pacc",
                                            name=f"pp{sb}_{half}")
                        nc.tensor.matmul(
                            st[half][:], outT3[:, :, sb * P : (sb + 1) * P],
                            wpf_sb[:, :, half * 512 : (half + 1) * 512],
                            start=True, stop=True,
                            perf_mode=mybir.MatmulPerfMode.DoubleRow,
                        )
                    def evict():
                        dst = ot[:, half * 512 : (half + 1) * 512]
                        if (sb + half) % 2 == 0:
                            nc.scalar.mul(dst, st[half][:], 1.0 / 1024.0)
                        else:
                            nc.vector.tensor_scalar_mul(
                                dst, st[half][:], 1.0 / 1024.0)
                    return [mm, evict]
                def dma():
                    nc.sync.dma_start(
                        out=pout[sb * P : (sb + 1) * P, :], in_=ot[:]
                    )
                return mk(0) + mk(1) + [dma]

            # ---- side-work queue -----------------------------------------
            side = []           # list of (tag, thunk); tag may be None
            done_tags = set()

            def _pop_one():
                tag, t = side.pop(0)
                t()
                if tag is not None:
                    done_tags.add(tag)

            def ensure(tag):
                while tag not in done_tags:
                    assert side, f"dependency {tag} not in side queue"
                    _pop_one()

            side_debt = [0.0]

            def pull_side(slots_left):
                # fractional pacing: spread the queue evenly over the
                # remaining slots instead of draining it early.
                if not side:
                    side_debt[0] = 0.0
                    return
                if slots_left <= 0:
                    while side:
                        _pop_one()
                    return
                side_debt[0] += len(side) / slots_left
                while side_debt[0] >= 1.0 and side:
                    _pop_one()
                    side_debt[0] -= 1.0

            def push_chain(tag, thunks):
                for i, t in enumerate(thunks):
                    side.append((tag if i == len(thunks) - 1 else None, t))

            state = {"pv": [], "pmul": None}

            # ---- softmax normalization chain -----------------------------
            def emit_norm(g2, qb, po):
                # immediate: one [65,1024] copy frees po; the denominators
                # go through an SBUF->SBUF reshape into [128,8] so the
                # reciprocal runs on all lanes, then one DRAM hop for the
                # partition broadcast; deferred: the two outT multiplies.
                oc = sbR.tile([65, 1024], f32, tag="ocopy",
                              name=f"oc{g2}_{qb}")
                if g2 == 1 and qb == NQB - 1:
                    # final block: latency-critical, split across engines
                    # (ACT is idle after the last exp).
                    nc.vector.tensor_copy(oc[:, 0:512], po[:, 0:512])
                    nc.scalar.copy(oc[:, 512:1024], po[:, 512:1024])
                else:
                    nc.vector.tensor_copy(oc[:], po[:])
                d128 = sbR.tile([P, 8], f32, tag="d128",
                                name=f"d128_{g2}_{qb}")
                nc.sync.dma_start(out=d128[:], in_=oc[64:65, :])
                # bf16 reciprocal: halves the 256KB broadcast transfer that
                # was a ~3.5us serial cost at the tail (recip error ~0.4% on
                # a value that only scales the small attention contribution).
                rec = sbR.tile([P, 8], bf16, tag="rec", name=f"rec{g2}_{qb}")
                with nc.allow_low_precision(reason="softmax recip in bf16"):
                    nc.vector.reciprocal(rec[:], d128[:])
                d2 = drS.tile([1, 1024], bf16, tag="d2", name=f"d2_{g2}_{qb}")
                nc.sync.dma_start(out=d2[:], in_=rec[:])
                bc = sbR.tile([64, 1024], bf16, tag="bcast",
                              name=f"bc{g2}_{qb}")
                nc.sync.dma_start(out=bc[:], in_=d2[:].partition_broadcast(64))

                def muls():
                    for h in range(2):
                        nc.vector.tensor_mul(
                            outT3[h * 64 : (h + 1) * 64, g2,
                                  qb * 512 : (qb + 1) * 512],
                            oc[0:64, h * 512 : (h + 1) * 512],
                            bc[:, h * 512 : (h + 1) * 512],
                        )
                state["pmul"] = muls

            # attnv rides DEPTH slots behind its scores so the exp engines
            # have extra slack before the PE queue blocks on them.
            FLUSH_DEPTH = 2

            def flush_pending(force=False):
                limit = 0 if force else FLUSH_DEPTH - 1
                while len(state["pv"]) > limit:
                    pg2, pqb, pkb, ppo, pat = state["pv"].pop(0)
                    for h in range(2):
                        nc.tensor.matmul(
                            ppo[:, h * 512 : (h + 1) * 512],
                            vaug[pg2][:, pkb, h, :],
                            pat[:, h * 512 : (h + 1) * 512],
                            start=(pkb == 0), stop=(pkb == NKB - 1),
                        )
                    if pkb == NKB - 1:
                        emit_norm(pg2, pqb, ppo)

            # ---- one attention block (g2, qb): 16 kb slots ----------------
            def attention_block(g2, qb):
                dve_kbs = DVE_KBS_ODD if qb % 2 else DVE_KBS_EVEN
                po = psO.tile([65, 1024], f32, tag="o",
                              name=f"po{g2}_{qb}")
                for kb in range(NKB):
                    ensure(("k", g2, kb // 4))
                    ensure(("v", kb))
                    # fire the deferred norm muls on an ACT-exp slot so the
                    # DVE ops don't delay a DVE-exp (kb 2 is DVE on even qb).
                    mul_kb = 3 if qb % 2 == 0 else 2
                    if kb == mul_kb and state["pmul"] is not None:
                        state["pmul"]()
                        state["pmul"] = None
                    # side work rides BEFORE the flush in every slot: the
                    # flushed attnv waits on this slot's exp, and side
                    # matmuls placed ahead of it in the PE queue hide that
                    # latency (also covers the po release at block starts).
                    slots_left = (NQB * 2 - 1 - (qb * 2 + g2)) * NKB \
                        + (NKB - 1 - kb)
                    if kb in (1, 2, 3):
                        pull_side(slots_left + 6)
                    ps = psS.tile([P, 1024], f32, tag="s",
                                  name=f"ps{g2}_{qb}_{kb}")
                    at = sbA.tile([P, 1024], bf16, tag="attnT",
                                  name=f"at{g2}_{qb}_{kb}")
                    for h in range(2):
                        nc.tensor.matmul(
                            ps[:, h * 512 : (h + 1) * 512],
                            kT[g2][h * 64 : (h + 1) * 64,
                                   kb * P : (kb + 1) * P],
                            qT[g2][h * 64 : (h + 1) * 64,
                                   qb * 512 : (qb + 1) * 512],
                            start=True, stop=True,
                            tile_position=(h * 64, 0),
                        )
                    if kb in dve_kbs:
                        nc.vector.tensor_scalar(
                            at[:].bitcast(i16), ps[:], EXP_A, EXP_B,
                            mybir.AluOpType.mult, mybir.AluOpType.add,
                        )
                    else:
                        nc.scalar.activation(
                            at[:], ps[:], mybir.ActivationFunctionType.Exp,
                            scale=ACT_SCALE,
                        )
                    if kb not in (1, 2, 3):
                        pull_side(slots_left)
                    flush_pending()
                    state["pv"].append((g2, qb, kb, po, at))

            # ---- emission schedule ---------------------------------------
            # prefix: q(0,0) + k(0,0) chains interleaved chunk-wise (track
            # DMA arrival), then v(0).
            qch = qk_chain_thunks(qT, 0, 0, 0)
            kch = qk_chain_thunks(kT, 1, 0, 0)
            for a, b in zip(qch, kch):
                a()
                b()
            for t in v_chain_thunks(0):
                t()
            done_tags.add(("q", 0, 0))
            done_tags.add(("k", 0, 0))
            done_tags.add(("v", 0))

            # side queue, in dependency-need order for block (0,0) then
            # block (1,0), then the rest.
            for j in (1, 2):
                push_chain(("v", j), v_chain_thunks(j))
            push_chain(("k", 0, 1), qk_chain_thunks(kT, 1, 0, 1))
            for j in (3, 4, 5):
                push_chain(("v", j), v_chain_thunks(j))
            push_chain(("k", 0, 2), qk_chain_thunks(kT, 1, 0, 2))
            for j in (6, 7):
                push_chain(("v", j), v_chain_thunks(j))
            push_chain(("q", 1, 0), qk_chain_thunks(qT, 0, 1, 0))
            for j in (8, 9):
                push_chain(("v", j), v_chain_thunks(j))
            push_chain(("k", 0, 3), qk_chain_thunks(kT, 1, 0, 3))
            for j in (10, 11):
                push_chain(("v", j), v_chain_thunks(j))
            push_chain(("k", 1, 0), qk_chain_thunks(kT, 1, 1, 0))
            for j in (12, 13):
                push_chain(("v", j), v_chain_thunks(j))
            push_chain(("k", 1, 1), qk_chain_thunks(kT, 1, 1, 1))
            for j in (14, 15):
                push_chain(("v", j), v_chain_thunks(j))
            push_chain(("k", 1, 2), qk_chain_thunks(kT, 1, 1, 2))
            push_chain(("k", 1, 3), qk_chain_thunks(kT, 1, 1, 3))
            push_chain(("q", 0, 1), qk_chain_thunks(qT, 0, 0, 1))
            push_chain(("q", 1, 1), qk_chain_thunks(qT, 0, 1, 1))
            push_chain(("q", 0, 2), qk_chain_thunks(qT, 0, 0, 2))
            push_chain(("q", 1, 2), qk_chain_thunks(qT, 0, 1, 2))
            push_chain(("q", 0, 3), qk_chain_thunks(qT, 0, 0, 3))
            push_chain(("q", 1, 3), qk_chain_thunks(qT, 0, 1, 3))

            # attention blocks: qb-major, alternating g2.  The norm muls for
            # (1, qb-1) fire at slot 2 of block (0, qb); proj group qb-1 is
            # released right after that block so it rides in (1, qb)'s slack.
            for qb in range(NQB):
                for g2 in range(2):
                    if qb or g2:
                        ensure(("q", g2, qb))
                    attention_block(g2, qb)
                    if g2 == 0 and qb >= 1:
                        for sb in range(4 * (qb - 1), 4 * qb):
                            push_chain(None, proj_thunks(sb))

            flush_pending(force=True)
            if state["pmul"] is not None:
                state["pmul"]()
                state["pmul"] = None
            # keep the PE (and HAM) warm through the final norm round trip
            pscr2 = psA.tile([P, 512], f32, tag="pacc", name="pwarm2")
            for _ in range(34):
                nc.tensor.matmul(pscr2[:], wscr[:, 0:P], wscr[:],
                                 start=True, stop=True)
            while side:
                _pop_one()
            # tail: last proj group emitted densely, psum from the freed
            # score pool so all four blocks pipeline; evictions split
            # across both engines.
            for sb in range(12, 16):
                pp = psS.tile([P, 1024], f32, tag="s", name=f"tp{sb}")
                for half in range(2):
                    nc.tensor.matmul(
                        pp[:, half * 512 : (half + 1) * 512],
                        outT3[:, :, sb * P : (sb + 1) * P],
                        wpf_sb[:, :, half * 512 : (half + 1) * 512],
                        start=True, stop=True,
                        perf_mode=mybir.MatmulPerfMode.DoubleRow,
                    )
                ot = sbP.tile([P, D], bf16, tag="pout", name=f"tot{sb}")
                nc.scalar.mul(ot[:, 0:512], pp[:, 0:512], 1.0 / 1024.0)
                nc.vector.tensor_scalar_mul(ot[:, 512:1024],
                                            pp[:, 512:1024], 1.0 / 1024.0)
                nc.sync.dma_start(out=pout[sb * P : (sb + 1) * P, :],
                                  in_=ot[:])

    _split_multi_waits(nc)
    return nc


def _chunk_major(a):
    # [1024, cols] -> [128, 8, cols]: element (p, c, j) = a[c*128 + p, j]
    return np.ascontiguousarray(
        a.reshape(NDMC, P, a.shape[1]).transpose(1, 0, 2))


def make_in_maps(x, Wq, Wk, Wv, Wp):
    import ml_dtypes

    bf = ml_dtypes.bfloat16
    fp8 = ml_dtypes.float8_e4m3
    x = np.ascontiguousarray(x, dtype=np.float32)
    Wq = np.asarray(Wq, dtype=np.float32)
    Wk = np.asarray(Wk, dtype=np.float32)
    Wv = np.asarray(Wv, dtype=np.float32)
    Wp = np.asarray(Wp, dtype=np.float32)
    ones_np = np.ones((1, P), dtype=np.float32)
    in_maps = []
    for c in range(NCORES):
        b, g = divmod(c, 4)
        cs = slice(g * GCOLS, (g + 1) * GCOLS)
        wqk = np.concatenate([32.0 * Wq[:, cs], 32.0 * Wk[:, cs]], axis=1)
        wp32 = (32.0 * Wp[cs, :]).reshape(2, P, D).transpose(1, 0, 2)
        in_maps.append(
            {
                "xtf": _chunk_major(x[b].T).astype(fp8),
                "wqkf": _chunk_major(wqk).astype(fp8),
                "wvf": _chunk_major(32.0 * Wv[:, cs]).astype(fp8),
                "wpf": np.ascontiguousarray(wp32).astype(fp8),
                "ones": ones_np,
            }
        )
    return in_maps


def kernel(x, Wq, Wk, Wv, Wp):
    global _last_results
    from concourse.bass_utils import run_bass_kernel_spmd

    x = np.ascontiguousarray(x, dtype=np.float32)

    if "nc" not in _cache:
        _cache["nc"] = _build()
    nc = _cache["nc"]

    in_maps = make_in_maps(x, Wq, Wk, Wv, Wp)
    res = run_bass_kernel_spmd(nc, in_maps, core_ids=list(range(NCORES)))
    _last_results = res

    out = np.empty((B, N, D), dtype=np.float32)
    for b in range(B):
        acc = x[b].copy()
        for g in range(4):
            acc += res.results[b * 4 + g]["pout"].astype(np.float32)
        out[b] = acc
    return out


# revision 85
# speedup vs baseline: 1.0275x; 1.0275x over previous
"""Multi-head attention (B=2, N=2048, D=1024, H=16, d_k=d_v=64) on 8
TRN2 NeuronCores.

Sharding: data parallel over batch (2) x tensor parallel over head
groups (4 heads per core). Each core computes its 4 heads' attention
plus the partial output projection (Wp rows for those heads); the host
sums the 4 partials per batch and adds the residual.

Design notes (vs the 257us baseline; ~186us measured):
  - attnv matmuls ride TWO slots behind their scores (FLUSH_DEPTH=2) so
    the exp engines have a full extra slot of slack before the in-order
    PE queue can block on them — worth ~20us alone.
  - the output projection also runs fp8 DoubleRow: outT is stored fp8
    as [dim, g2, seq] so one matmul contracts both 128-dim chunks; Wp is
    host-prescaled by 32 and evictions scale by 1/1024.
  - q/k/v projections run in fp8e4m3 with DoubleRow (contraction 256 per
    matmul, chunk-pair 3D APs over [128, 8, cols] tiles), halving their
    matmul count; weights are host-prescaled by 32 to dodge e4m3
    denormals and Wp is pre-divided by 32 to compensate.
  - exp is split between ScalarE (true exp, 10/16 slots) and VectorE
    (Schraudolph int16-bit-trick exp: bits = round(A*psum + 16250.4)
    viewed as bf16, ~3% max rel err, 6/16 slots, engine-alternating) so
    ACT is no longer a ~143us serial bottleneck.
  - softmax reciprocal: denominators are staged through an SBUF->SBUF
    reshape DMA into [128, 8] so one 128-lane reciprocal replaces the
    pathological [1,512] 1-lane reciprocals (53us -> ~1us), then one
    DRAM hop for the partition broadcast.
  - loads shrink to 3.25MB (fp8), issued from BOTH sync and scalar DMA
    queues; dummy matmuls on scratch SBUF keep the PE busy through the
    load wait so HAM un-throttles to 2.4 GHz by ~11us.
  - attention blocks alternate g2 (head-pair) per qb so projection and
    normalization spread evenly; proj matmuls and leftover chains ride
    in the slack via a fractionally-paced side queue.
"""
import numpy as np

import concourse.bass as bass
import concourse.tile as tile
from concourse import mybir
from concourse.vector_clock import ScopedClock

f32 = mybir.dt.float32
f32r = mybir.dt.float32r
bf16 = mybir.dt.bfloat16
i16 = mybir.dt.int16
f8 = mybir.dt.float8e4

B, N, D = 2, 2048, 1024
H, DK = 16, 64
HPC = 4          # heads per core
GCOLS = HPC * DK  # 256 weight columns per core
NCORES = 8
P = 128
NKB = N // P      # 16 key blocks
NQB = N // 512    # 4 query blocks of 512
NDMC = D // P     # 8 d_model chunks

# q/k/v weights and x are fed to the PE in fp8e4m3 with the weights
# pre-scaled by 32 (avoids e4m3 denormals); scores come out as
# 1024*(q.k) and Wp is pre-divided by 32 to undo the v scaling.
QK_SCALE = 1024.0
# Schraudolph fast-exp constants (DVE int16 trick), calibrated on HW:
# bits = round_i16(EXP_A * psum + EXP_B); bits viewed as bf16 ~= exp(s/8)
EXP_A = 0.125 * 1.4426950408889634 * 128.0 / QK_SCALE
EXP_B = 16256.0 - 5.6
ACT_SCALE = 0.125 / QK_SCALE
# slots (by kb) whose exp runs on DVE instead of ACT, alternating so
# consecutive slots' exps pipeline across the two engines; kb 0-1 stay
# on ACT so DVE is free to release the po psum at block boundaries.
DVE_KBS_EVEN = frozenset({2, 4, 6, 8, 10, 12})
DVE_KBS_ODD = frozenset({3, 5, 7, 9, 11, 13})

_cache = {}
_last_results = None


# ---------------------------------------------------------------------------
# Workarounds for this walrus build: max ONE sync wait per instruction.
# ---------------------------------------------------------------------------
_ws_counter = [0]


def _split_multi_waits(nc, limit=1):
    for f in nc.m.functions:
        for bb in f.blocks:
            new = []
            changed = False
            for inst in bb.instructions:
                si = inst.sync_info
                waits = list(si.on_wait) if si is not None and si.on_wait else []
                if len(waits) > limit:
                    changed = True
                    extra = waits[:-limit]
                    for i in range(0, len(extra), limit):
                        _ws_counter[0] += 1
                        nop = mybir.InstNoOp(
                            name=f"I-waitsplit-{_ws_counter[0]}", ins=[], outs=[]
                        )
                        nop.engine = inst.engine
                        nop.sync_info = mybir.SyncInfo(
                            on_wait=extra[i : i + limit], on_update=[]
                        )
                        new.append(nop)
                    si.on_wait = waits[-limit:]
                    inst.sync_info = si
                new.append(inst)
            if changed:
                bb.instructions = new


def _patched_drain_and_barrier(self, tick_clock, wait_clock):
    nc = self.nc
    drain_inst = nc.sync.drain()
    wait_clock.add_sem_waits(
        drain_inst.ins, ScopedClock({None: tick_clock.global_clock})
    )
    si = drain_inst.ins.sync_info
    if si is not None and si.on_wait is not None and len(si.on_wait) > 1:
        waits = list(si.on_wait)
        si.on_wait = waits[:1]
        drain_inst.ins.sync_info = si
        for i in range(1, len(waits)):
            extra = nc.sync.drain()
            esi = extra.ins.sync_info
            if esi is None:
                esi = mybir.SyncInfo(on_wait=[], on_update=[])
            esi.on_wait = waits[i : i + 1]
            extra.ins.sync_info = esi
    nc.all_engine_barrier()
    assert self.sems is not None
    popped = nc._tile_sem_poison_stack.pop()
    assert popped is self._sem_poison
    nc.clear_and_free_semaphores(list(self.sems.allocated().values()))
    nc.all_engine_barrier()


tile.TileContext._drain_and_barrier = _patched_drain_and_barrier


# ---------------------------------------------------------------------------
# Kernel build
# ---------------------------------------------------------------------------
def _build():
    nc = bass.Bass()
    # fp8 operands, chunk-major [128, 8 d-chunks, cols] for DoubleRow
    xtf = nc.dram_tensor("xtf", [P, NDMC, N], f8, kind="ExternalInput")
    wqkf = nc.dram_tensor("wqkf", [P, NDMC, 2 * GCOLS], f8,
                          kind="ExternalInput")
    wvf = nc.dram_tensor("wvf", [P, NDMC, GCOLS], f8, kind="ExternalInput")
    wpf = nc.dram_tensor("wpf", [P, 2, D], f8, kind="ExternalInput")
    ones = nc.dram_tensor("ones", [1, P], f32r, kind="ExternalInput")
    pout = nc.dram_tensor("pout", [N, D], bf16, kind="ExternalOutput")

    with tile.TileContext(nc) as tc:
        import contextlib

        with contextlib.ExitStack() as ctx:
            sbX = ctx.enter_context(tc.tile_pool(name="sbX", bufs=1))
            sbW = ctx.enter_context(tc.tile_pool(name="sbW", bufs=1))
            sbQK = ctx.enter_context(tc.tile_pool(name="sbQK", bufs=1))
            sbV = ctx.enter_context(tc.tile_pool(name="sbV", bufs=1))
            sbO = ctx.enter_context(tc.tile_pool(name="sbO", bufs=1))
            sbA = ctx.enter_context(tc.tile_pool(name="sbA", bufs=6))
            sbR = ctx.enter_context(tc.tile_pool(name="sbR", bufs=3))
            sbP = ctx.enter_context(tc.tile_pool(name="sbP", bufs=3))
            drS = ctx.enter_context(tc.tile_pool(name="drS", bufs=4,
                                                  space="DRAM"))
            psS = ctx.enter_context(tc.tile_pool(name="psS", bufs=2, space="PSUM"))
            psO = ctx.enter_context(tc.tile_pool(name="psO", bufs=1, space="PSUM"))
            psA = ctx.enter_context(tc.tile_pool(name="psA", bufs=2, space="PSUM"))

            # ---- loads ----------------------------------------------------
            # sync engine: ones, wqkf pairs, xtf col-waves 0 and 2.
            # scalar engine: wvf, xtf col-wave 1, warm, wp.
            ones_sb = sbW.tile([1, P], f32r, tag="ones")
            nc.sync.dma_start(out=ones_sb[:], in_=ones[:])

            wqkf_sb = sbW.tile([P, NDMC, 2 * GCOLS], f8, tag="wqkf")
            for cc in range(4):
                nc.sync.dma_start(out=wqkf_sb[:, 2 * cc : 2 * cc + 2, :],
                                  in_=wqkf[:, 2 * cc : 2 * cc + 2, :])
            xtf_sb = sbX.tile([P, NDMC, N], f8, tag="xtf")
            for cc in range(4):
                nc.sync.dma_start(
                    out=xtf_sb[:, 2 * cc : 2 * cc + 2, 0:512],
                    in_=xtf[:, 2 * cc : 2 * cc + 2, 0:512])

            wvf_sb = sbW.tile([P, NDMC, GCOLS], f8, tag="wvf")
            for half in range(2):
                nc.scalar.dma_start(out=wvf_sb[:, 4 * half : 4 * half + 4, :],
                                    in_=wvf[:, 4 * half : 4 * half + 4, :])
            for cc in range(4):
                nc.scalar.dma_start(
                    out=xtf_sb[:, 2 * cc : 2 * cc + 2, 512:1024],
                    in_=xtf[:, 2 * cc : 2 * cc + 2, 512:1024])

            # Pre-warm the exp table (~2.7us ACT table load) AFTER the hot
            # scalar-queue loads so it doesn't delay them, but well before
            # the first real exp.
            warm = sbR.tile([1, 2], f32, tag="warm")
            nc.scalar.activation(
                warm[:], ones_sb[0:1, 0:2], mybir.ActivationFunctionType.Exp
            )

            for cc in range(4):
                nc.sync.dma_start(
                    out=xtf_sb[:, 2 * cc : 2 * cc + 2, 1024:N],
                    in_=xtf[:, 2 * cc : 2 * cc + 2, 1024:N])

            wpf_sb = sbW.tile([P, 2, D], f8, tag="wpf")
            for g2 in range(2):
                nc.scalar.dma_start(out=wpf_sb[:, g2 : g2 + 1, :],
                                    in_=wpf[:, g2 : g2 + 1, :])

            # Dummy matmuls on scratch SBUF keep the PE busy through the
            # load wait so HAM un-throttles (K=8/8) before the real chains;
            # without them everything before ~20us runs at 1.2 GHz.
            wscr = sbW.tile([P, 512], bf16, tag="wscr")
            nc.vector.memset(wscr[:], 1.0)
            pscr = psA.tile([P, 512], f32, tag="pacc", name="pwarm")
            for _ in range(32):
                nc.tensor.matmul(pscr[:], wscr[:, 0:P], wscr[:],
                                 start=True, stop=True)

            qT = [sbQK.tile([P, N], bf16, tag=f"qT{g2}", name=f"qT{g2}")
                  for g2 in range(2)]
            kT = [sbQK.tile([P, N], bf16, tag=f"kT{g2}", name=f"kT{g2}")
                  for g2 in range(2)]
            vaug = [sbV.tile([P, NKB, 2, 65], bf16, tag=f"vaug{g2}",
                             name=f"vaug{g2}") for g2 in range(2)]
            for g2 in range(2):
                nc.vector.memset(vaug[g2][:, :, :, 64:65], 1.0)
            # normalized attention output, fp8, [dim-of-g2, g2, seq] so the
            # proj matmul can pair the two g2 chunks with DoubleRow.
            outT3 = sbO.tile([P, 2, N], f8, tag="outT3", name="outT3")

            # ---- chain thunk builders (fp8 DoubleRow: 4 chunk-pair MMs) --
            def qk_chain_thunks(dst, which, g2, qb):
                # which: 0 = wq (cols 0:256), 1 = wk (cols 256:512)
                base = which * GCOLS + g2 * P
                st = {}
                def mm(cc):
                    if cc == 0:
                        st["p"] = psA.tile([P, 512], f32, tag="pacc",
                                           name=f"pqk{which}_{g2}_{qb}")
                    nc.tensor.matmul(
                        st["p"][:],
                        wqkf_sb[:, 2 * cc : 2 * cc + 2, base : base + P],
                        xtf_sb[:, 2 * cc : 2 * cc + 2,
                               qb * 512 : (qb + 1) * 512],
                        start=(cc == 0), stop=(cc == 3),
                        perf_mode=mybir.MatmulPerfMode.DoubleRow,
                    )
                def evict():
                    nc.vector.tensor_copy(
                        dst[g2][:, qb * 512 : (qb + 1) * 512], st["p"][:]
                    )
                return [lambda cc=cc: mm(cc) for cc in range(4)] + [evict]

            def v_chain_thunks(kb):
                st = {}
                def mm(cc):
                    if cc == 0:
                        st["p"] = psA.tile([P, 2, P], f32, tag="pacc",
                                           name=f"pv{kb}")
                    nc.tensor.matmul(
                        st["p"][:],
                        xtf_sb[:, 2 * cc : 2 * cc + 2, kb * P : (kb + 1) * P],
                        wvf_sb[:, 2 * cc : 2 * cc + 2, :],
                        start=(cc == 0), stop=(cc == 3),
                        perf_mode=mybir.MatmulPerfMode.DoubleRow,
                    )
                def ev(g2):
                    nc.vector.tensor_copy(
                        vaug[g2][:, kb, :, 0:64], st["p"][:, g2, :]
                    )
                return ([lambda cc=cc: mm(cc) for cc in range(4)]
                        + [lambda: ev(0), lambda: ev(1)])

            def proj_thunks(sb):
                # per seq block sb: one DoubleRow matmul per 512-col half
                # (contracts both g2 chunks at once); psum holds 1024*pout
                # so the eviction scales by 1/1024.  Evictions split ACT/DVE.
                ot = sbP.tile([P, D], bf16, tag="pout", name=f"ot{sb}")
                st = {}
                def mk(half):
                    def mm():
                        st[half] = psA.tile([P, 512], f32, tag="pacc",
                                            name=f"pp{sb}_{half}")
                        nc.tensor.matmul(
                            st[half][:], outT3[:, :, sb * P : (sb + 1) * P],
                            wpf_sb[:, :, half * 512 : (half + 1) * 512],
                            start=True, stop=True,
                            perf_mode=mybir.MatmulPerfMode.DoubleRow,
                        )
                    def evict():
                        dst = ot[:, half * 512 : (half + 1) * 512]
                        if (sb + half) % 2 == 0:
                            nc.scalar.mul(dst, st[half][:], 1.0 / 1024.0)
                        else:
                            nc.vector.tensor_scalar_mul(
                                dst, st[half][:], 1.0 / 1024.0)
                    return [mm, evict]
                def dma():
                    nc.sync.dma_start(
                        out=pout[sb * P : (sb + 1) * P, :], in_=ot[:]
                    )
                return mk(0) + mk(1) + [dma]

            # ---- side-work queue -----------------------------------------
            side = []           # list of (tag, thunk); tag may be None
            done_tags = set()

            def _pop_one():
                tag, t = side.pop(0)
                t()
                if tag is not None:
                    done_tags.add(tag)

            def ensure(tag):
                while tag not in done_tags:
                    assert side, f"dependency {tag} not in side queue"
                    _pop_one()

            side_debt = [0.0]

            def pull_side(slots_left):
                # fractional pacing: spread the queue evenly over the
                # remaining slots instead of draining it early.
                if not side:
                    side_debt[0] = 0.0
                    return
                if slots_left <= 0:
                    while side:
                        _pop_one()
                    return
                side_debt[0] += len(side) / slots_left
                while side_debt[0] >= 1.0 and side:
                    _pop_one()
                    side_debt[0] -= 1.0

            def push_chain(tag, thunks):
                for i, t in enumerate(thunks):
                    side.append((tag if i == len(thunks) - 1 else None, t))

            state = {"pv": [], "pmul": None}

            # ---- softmax normalization chain -----------------------------
            def emit_norm(g2, qb, po):
                # immediate: one [65,1024] copy frees po; the denominators
                # go through an SBUF->SBUF reshape into [128,8] so the
                # reciprocal runs on all lanes, then one DRAM hop for the
                # partition broadcast; deferred: the two outT multiplies.
                oc = sbR.tile([65, 1024], f32, tag="ocopy",
                              name=f"oc{g2}_{qb}")
                if g2 == 1 and qb == NQB - 1:
                    # final block: latency-critical, split across engines
                    # (ACT is idle after the last exp).
                    nc.vector.tensor_copy(oc[:, 0:512], po[:, 0:512])
                    nc.scalar.copy(oc[:, 512:1024], po[:, 512:1024])
                else:
                    nc.vector.tensor_copy(oc[:], po[:])
                d128 = sbR.tile([P, 8], f32, tag="d128",
                                name=f"d128_{g2}_{qb}")
                nc.sync.dma_start(out=d128[:], in_=oc[64:65, :])
                rec = sbR.tile([P, 8], f32, tag="rec", name=f"rec{g2}_{qb}")
                nc.vector.reciprocal(rec[:], d128[:])
                d2 = drS.tile([1, 1024], f32, tag="d2", name=f"d2_{g2}_{qb}")
                nc.sync.dma_start(out=d2[:], in_=rec[:])
                bc = sbR.tile([64, 1024], f32, tag="bcast",
                              name=f"bc{g2}_{qb}")
                nc.sync.dma_start(out=bc[:], in_=d2[:].partition_broadcast(64))

                def muls():
                    for h in range(2):
                        nc.vector.tensor_mul(
                            outT3[h * 64 : (h + 1) * 64, g2,
                                  qb * 512 : (qb + 1) * 512],
                            oc[0:64, h * 512 : (h + 1) * 512],
                            bc[:, h * 512 : (h + 1) * 512],
                        )
                state["pmul"] = muls

            # attnv rides DEPTH slots behind its scores so the exp engines
            # have extra slack before the PE queue blocks on them.
            FLUSH_DEPTH = 2

            def flush_pending(force=False):
                limit = 0 if force else FLUSH_DEPTH - 1
                while len(state["pv"]) > limit:
                    pg2, pqb, pkb, ppo, pat = state["pv"].pop(0)
                    for h in range(2):
                        nc.tensor.matmul(
                            ppo[:, h * 512 : (h + 1) * 512],
                            vaug[pg2][:, pkb, h, :],
                            pat[:, h * 512 : (h + 1) * 512],
                            start=(pkb == 0), stop=(pkb == NKB - 1),
                        )
                    if pkb == NKB - 1:
                        emit_norm(pg2, pqb, ppo)

            # ---- one attention block (g2, qb): 16 kb slots ----------------
            def attention_block(g2, qb):
                dve_kbs = DVE_KBS_ODD if qb % 2 else DVE_KBS_EVEN
                po = psO.tile([65, 1024], f32, tag="o",
                              name=f"po{g2}_{qb}")
                for kb in range(NKB):
                    ensure(("k", g2, kb // 4))
                    ensure(("v", kb))
                    # fire the deferred norm muls on an ACT-exp slot so the
                    # DVE ops don't delay a DVE-exp (kb 2 is DVE on even qb).
                    mul_kb = 3 if qb % 2 == 0 else 2
                    if kb == mul_kb and state["pmul"] is not None:
                        state["pmul"]()
                        state["pmul"] = None
                    # side work rides BEFORE the flush in every slot: the
                    # flushed attnv waits on this slot's exp, and side
                    # matmuls placed ahead of it in the PE queue hide that
                    # latency (also covers the po release at block starts).
                    slots_left = (NQB * 2 - 1 - (qb * 2 + g2)) * NKB \
                        + (NKB - 1 - kb)
                    if kb in (1, 2, 3):
                        pull_side(slots_left + 6)
                    ps = psS.tile([P, 1024], f32, tag="s",
                                  name=f"ps{g2}_{qb}_{kb}")
                    at = sbA.tile([P, 1024], bf16, tag="attnT",
                                  name=f"at{g2}_{qb}_{kb}")
                    for h in range(2):
                        nc.tensor.matmul(
                            ps[:, h * 512 : (h + 1) * 512],
                            kT[g2][h * 64 : (h + 1) * 64,
                                   kb * P : (kb + 1) * P],
                            qT[g2][h * 64 : (h + 1) * 64,
                                   qb * 512 : (qb + 1) * 512],
                            start=True, stop=True,
                            tile_position=(h * 64, 0),
                        )
                    if kb in dve_kbs:
                        nc.vector.tensor_scalar(
                            at[:].bitcast(i16), ps[:], EXP_A, EXP_B,
                            mybir.AluOpType.mult, mybir.AluOpType.add,
                        )
                    else:
                        nc.scalar.activation(
                            at[:], ps[:], mybir.ActivationFunctionType.Exp,
                            scale=ACT_SCALE,
                        )
                    if kb not in (1, 2, 3):
                        pull_side(slots_left)
                    flush_pending()
                    state["pv"].append((g2, qb, kb, po, at))

            # ---- emission schedule ---------------------------------------
            # prefix: q(0,0) + k(0,0) chains interleaved chunk-wise (track
            # DMA arrival), then v(0).
            qch = qk_chain_thunks(qT, 0, 0, 0)
            kch = qk_chain_thunks(kT, 1, 0, 0)
            for a, b in zip(qch, kch):
                a()
                b()
            for t in v_chain_thunks(0):
                t()
            done_tags.add(("q", 0, 0))
            done_tags.add(("k", 0, 0))
            done_tags.add(("v", 0))

            # side queue, in dependency-need order for block (0,0) then
            # block (1,0), then the rest.
            for j in (1, 2):
                push_chain(("v", j), v_chain_thunks(j))
            push_chain(("k", 0, 1), qk_chain_thunks(kT, 1, 0, 1))
            for j in (3, 4, 5):
                push_chain(("v", j), v_chain_thunks(j))
            push_chain(("k", 0, 2), qk_chain_thunks(kT, 1, 0, 2))
            for j in (6, 7):
                push_chain(("v", j), v_chain_thunks(j))
            push_chain(("q", 1, 0), qk_chain_thunks(qT, 0, 1, 0))
            for j in (8, 9):
                push_chain(("v", j), v_chain_thunks(j))
            push_chain(("k", 0, 3), qk_chain_thunks(kT, 1, 0, 3))
            for j in (10, 11):
                push_chain(("v", j), v_chain_thunks(j))
            push_chain(("k", 1, 0), qk_chain_thunks(kT, 1, 1, 0))
            for j in (12, 13):
                push_chain(("v", j), v_chain_thunks(j))
            push_chain(("k", 1, 1), qk_chain_thunks(kT, 1, 1, 1))
            for j in (14, 15):
                push_chain(("v", j), v_chain_thunks(j))
            push_chain(("k", 1, 2), qk_chain_thunks(kT, 1, 1, 2))
            push_chain(("k", 1, 3), qk_chain_thunks(kT, 1, 1, 3))
            push_chain(("q", 0, 1), qk_chain_thunks(qT, 0, 0, 1))
            push_chain(("q", 1, 1), qk_chain_thunks(qT, 0, 1, 1))
            push_chain(("q", 0, 2), qk_chain_thunks(qT, 0, 0, 2))
            push_chain(("q", 1, 2), qk_chain_thunks(qT, 0, 1, 2))
            push_chain(("q", 0, 3), qk_chain_thunks(qT, 0, 0, 3))
            push_chain(("q", 1, 3), qk_chain_thunks(qT, 0, 1, 3))

            # attention blocks: qb-major, alternating g2.  The norm muls for
            # (1, qb-1) fire at slot 2 of block (0, qb); proj group qb-1 is
            # released right after that block so it rides in (1, qb)'s slack.
            for qb in range(NQB):
                for g2 in range(2):
                    if qb or g2:
                        ensure(("q", g2, qb))
                    attention_block(g2, qb)
                    if g2 == 0 and qb >= 1:
                        for sb in range(4 * (qb - 1), 4 * qb):
                            push_chain(None, proj_thunks(sb))

            flush_pending(force=True)
            if state["pmul"] is not None:
                state["pmul"]()
                state["pmul"] = None
            # keep the PE (and HAM) warm through the final norm round trip
            pscr2 = psA.tile([P, 512], f32, tag="pacc", name="pwarm2")
            for _ in range(34):
                nc.tensor.matmul(pscr2[:], wscr[:, 0:P], wscr[:],
                                 start=True, stop=True)
            while side:
                _pop_one()
            # tail: last proj group emitted densely, psum from the freed
            # score pool so all four blocks pipeline; evictions split
            # across both engines.
            for sb in range(12, 16):
                pp = psS.tile([P, 1024], f32, tag="s", name=f"tp{sb}")
                for half in range(2):
                    nc.tensor.matmul(
                        pp[:, half * 512 : (half + 1) * 512],
                        outT3[:, :, sb * P : (sb + 1) * P],
                        wpf_sb[:, :, half * 512 : (half + 1) * 512],
                        start=True, stop=True,
                        perf_mode=mybir.MatmulPerfMode.DoubleRow,
                    )
                ot = sbP.tile([P, D], bf16, tag="pout", name=f"tot{sb}")
                nc.scalar.mul(ot[:, 0:512], pp[:, 0:512], 1.0 / 1024.0)
                nc.vector.tensor_scalar_mul(ot[:, 512:1024],
                                            pp[:, 512:1024], 1.0 / 1024.0)
                nc.sync.dma_start(out=pout[sb * P : (sb + 1) * P, :],
                                  in_=ot[:])

    _split_multi_waits(nc)
    return nc


def _chunk_major(a):
    # [1024, cols] -> [128, 8, cols]: element (p, c, j) = a[c*128 + p, j]
    return np.ascontiguousarray(
        a.reshape(NDMC, P, a.shape[1]).transpose(1, 0, 2))


def make_in_maps(x, Wq, Wk, Wv, Wp):
    import ml_dtypes

    bf = ml_dtypes.bfloat16
    fp8 = ml_dtypes.float8_e4m3
    x = np.ascontiguousarray(x, dtype=np.float32)
    Wq = np.asarray(Wq, dtype=np.float32)
    Wk = np.asarray(Wk, dtype=np.float32)
    Wv = np.asarray(Wv, dtype=np.float32)
    Wp = np.asarray(Wp, dtype=np.float32)
    ones_np = np.ones((1, P), dtype=np.float32)
    in_maps = []
    for c in range(NCORES):
        b, g = divmod(c, 4)
        cs = slice(g * GCOLS, (g + 1) * GCOLS)
        wqk = np.concatenate([32.0 * Wq[:, cs], 32.0 * Wk[:, cs]], axis=1)
        wp32 = (32.0 * Wp[cs, :]).reshape(2, P, D).transpose(1, 0, 2)
        in_maps.append(
            {
                "xtf": _chunk_major(x[b].T).astype(fp8),
                "wqkf": _chunk_major(wqk).astype(fp8),
                "wvf": _chunk_major(32.0 * Wv[:, cs]).astype(fp8),
                "wpf": np.ascontiguousarray(wp32).astype(fp8),
                "ones": ones_np,
            }
        )
    return in_maps


def kernel(x, Wq, Wk, Wv, Wp):
    global _last_results
    from concourse.bass_utils import run_bass_kernel_spmd

    x = np.ascontiguousarray(x, dtype=np.float32)

    if "nc" not in _cache:
        _cache["nc"] = _build()
    nc = _cache["nc"]

    in_maps = make_in_maps(x, Wq, Wk, Wv, Wp)
    res = run_bass_kernel_spmd(nc, in_maps, core_ids=list(range(NCORES)))
    _last_results = res

    out = np.empty((B, N, D), dtype=np.float32)
    for b in range(B):
        acc = x[b].copy()
        for g in range(4):
            acc += res.results[b * 4 + g]["pout"].astype(np.float32)
        out[b] = acc
    return out
